# revision 1
# baseline (speedup 1.0000x reference)
"""Trainium2 Bass kernel for CausalSelfAttention (B=1, T=2048, C=4096,
32 heads / 8 query groups / head_size 128, full-dim RoPE, GQA).

Sharding: tensor-parallel over the 8 query groups. Core g owns w_attn rows
[g*768:(g+1)*768] (4 q heads + 1 k + 1 v) and w_proj columns
[g*512:(g+1)*512]; x is replicated. Each core returns a partial projection
output [2048, 4096]; the host sums the 8 partials (the all-reduce).

All matmuls run as float32r (full-rate fp32 PE mode, N=512).
"""

import os
import sys

for _p in ("/opt/trn_rl_repo", "/root/.axon_site/_ro/trn_rl_repo"):
    if os.path.isdir(_p) and _p not in sys.path:
        sys.path.insert(0, _p)

import numpy as np

import concourse.bass as bass
import concourse.mybir as mybir
import concourse.tile as tile
from concourse import bacc, bass_utils

N_CORES = 8
T = 2048
C = 4096
HS = 128
N_HEAD = 32
G = 8                      # query groups == cores
QPK = 4                    # q heads per group
NCOMP = QPK + 2            # q0..q3, k, v
RG = NCOMP * HS            # 768 w_attn rows per group
OG = QPK * HS              # 512 proj-input cols per group
NT = T // 512              # 4 blocks of 512 along t
NC = C // 128              # 32 contraction chunks
NKT = T // 128             # 16 k tiles
SCALE = 1.0 / np.sqrt(float(HS))

F32 = mybir.dt.float32
MMDT = mybir.dt.float32r   # matmul operand dtype


def _build_program():
    nc = bacc.Bacc(trn_type="TRN2", target_bir_lowering=False, debug=False,
                   num_devices=N_CORES)

    d_x = nc.dram_tensor("xt", [C, T], MMDT, kind="ExternalInput").ap()
    d_wa = nc.dram_tensor("wat", [C, RG], MMDT, kind="ExternalInput").ap()
    d_wp = nc.dram_tensor("wpt", [OG, C], MMDT, kind="ExternalInput").ap()
    d_cos = nc.dram_tensor("cost", [HS, T], MMDT, kind="ExternalInput").ap()
    d_sin = nc.dram_tensor("sint", [HS, T], MMDT, kind="ExternalInput").ap()
    d_mask = nc.dram_tensor("mask", [128, 4 * 512], MMDT, kind="ExternalInput").ap()
    d_perm = nc.dram_tensor("perm", [128, 128], MMDT, kind="ExternalInput").ap()
    d_idn = nc.dram_tensor("idn", [128, 128], MMDT, kind="ExternalInput").ap()
    d_ones = nc.dram_tensor("ones", [128, 128], MMDT, kind="ExternalInput").ap()
    d_out = nc.dram_tensor("out", [T, C], F32, kind="ExternalOutput").ap()

    with tile.TileContext(nc) as tc:
        with tc.tile_pool(name="glob", bufs=1) as glob:
            # roped q0..q3 / k, one tile per (comp, t-block): [hs=128, 512]
            QQ = [[glob.tile([128, 512], MMDT, name=f"qq{j}_{tb}",
                             tag=f"qq{j}_{tb}")
                   for tb in range(NT)] for j in range(5)]
            # V in [t, hs] layout, one tile per t-block: col u = t-chunk
            V = [glob.tile([128, 512], MMDT, name=f"v{tb}", tag=f"v{tb}")
                 for tb in range(NT)]
            ONES = glob.tile([128, 128], MMDT)
            PERM = glob.tile([128, 128], MMDT)
            IDN = glob.tile([128, 128], MMDT)

            # ---------------- Phase A: qkv projection + rope -------------
            with tc.tile_pool(name="wa", bufs=1) as wap, \
                 tc.tile_pool(name="csp", bufs=2) as csp, \
                 tc.tile_pool(name="xp", bufs=8) as xp, \
                 tc.tile_pool(name="tmpa", bufs=2) as tmpa, \
                 tc.tile_pool(name="psA", bufs=1, space="PSUM") as psA, \
                 tc.tile_pool(name="psR", bufs=2, space="PSUM") as psR:
                WA = [wap.tile([128, RG], MMDT, name=f"wa{n}", tag=f"wa{n}")
                      for n in range(NC)]

                for tb in range(NT):
                    ts = slice(tb * 512, (tb + 1) * 512)
                    qkv_ps = [psA.tile([128, 512], F32, tag=f"qkv{j}",
                                       name=f"qkv{j}")
                              for j in range(NCOMP)]
                    for n in range(NC):
                        if tb == 0:
                            # interleave weight and activation loads so the
                            # first matmuls start after ~one chunk of DMA
                            nc.sync.dma_start(WA[n][:],
                                              d_wa[n * 128:(n + 1) * 128, :])
                        xt = xp.tile([128, 512], MMDT, tag="x")
                        nc.sync.dma_start(xt[:], d_x[n * 128:(n + 1) * 128, ts])
                        for j in range(NCOMP):
                            nc.tensor.matmul(
                                qkv_ps[j][:],
                                WA[n][:, j * HS:(j + 1) * HS],
                                xt[:],
                                start=(n == 0), stop=(n == NC - 1))

                    if tb == 0:
                        nc.sync.dma_start(ONES[:], d_ones[:])
                        nc.sync.dma_start(PERM[:], d_perm[:])
                        nc.sync.dma_start(IDN[:], d_idn[:])
                    cost = csp.tile([128, 512], MMDT, tag="cos")
                    sint = csp.tile([128, 512], MMDT, tag="sin")
                    nc.sync.dma_start(cost[:], d_cos[:, ts])
                    nc.sync.dma_start(sint[:], d_sin[:, ts])

                    for j in range(5):  # q0..q3, k get rope
                        raw = tmpa.tile([128, 512], MMDT, tag="raw")
                        nc.scalar.copy(raw[:], qkv_ps[j][:])
                        rot = psR.tile([128, 512], F32, tag="rot")
                        nc.tensor.matmul(rot[:], PERM[:], raw[:],
                                         start=True, stop=True)
                        t1 = tmpa.tile([128, 512], MMDT, tag="t1")
                        nc.vector.tensor_tensor(t1[:], raw[:], cost[:],
                                                mybir.AluOpType.mult)
                        t2 = tmpa.tile([128, 512], MMDT, tag="t2")
                        nc.vector.tensor_tensor(t2[:], rot[:], sint[:],
                                                mybir.AluOpType.mult)
                        nc.vector.tensor_tensor(QQ[j][tb][:], t1[:], t2[:],
                                                mybir.AluOpType.add)

                    # v: transpose [hs, t] -> [t, hs] chunks
                    vraw = tmpa.tile([128, 512], MMDT, tag="raw")
                    nc.scalar.copy(vraw[:], qkv_ps[5][:])
                    for u in range(4):
                        vt = psR.tile([128, 128], MMDT, tag="rot")
                        nc.tensor.transpose(vt[:], vraw[:, u * 128:(u + 1) * 128],
                                            IDN[:])
                        nc.scalar.copy(V[tb][:, u * 128:(u + 1) * 128], vt[:])

            # ---------------- Phase B: causal attention ------------------
            with tc.tile_pool(name="wp", bufs=1) as wpp, \
                 tc.tile_pool(name="ptp", bufs=7) as ptp, \
                 tc.tile_pool(name="bcp", bufs=2) as bcp, \
                 tc.tile_pool(name="rcp", bufs=2) as rcp:
                MASK = wpp.tile([128, 4 * 512], MMDT)
                nc.sync.dma_start(MASK[:], d_mask[:])
                WP = wpp.tile([128, QPK * C], MMDT)
                for h in range(QPK):
                    nc.sync.dma_start(WP[:, h * C:(h + 1) * C],
                                      d_wp[h * 128:(h + 1) * 128, :])
                Y = [wpp.tile([128, T], MMDT, name=f"y{h}", tag=f"y{h}")
                     for h in range(QPK)]

                with tc.tile_pool(name="psS", bufs=4, space="PSUM") as psS, \
                     tc.tile_pool(name="psY", bufs=2, space="PSUM") as psY, \
                     tc.tile_pool(name="psD", bufs=1, space="PSUM") as psD, \
                     tc.tile_pool(name="psB", bufs=1, space="PSUM") as psB:
                    for h in range(QPK):
                        for b in range(NT):
                            nkt = 4 * (b + 1)
                            y_ps = psY.tile([128, 512], F32, tag="y")
                            d_ps = psD.tile([1, 512], F32, tag="d")
                            for kt in range(nkt):
                                r = kt - 4 * b
                                # triangular narrowing: diagonal tiles only
                                # compute live columns (N stays >=256 so
                                # float32r keeps its 1 cycle/row rate), and
                                # the mask multiply covers just the staircase
                                # strip instead of the full tile
                                off = 0 if r <= 0 else min(r, 2) * 128
                                if r < 0:
                                    moff, mw = None, 0
                                elif r == 0:
                                    moff, mw = 0, 128
                                elif r == 3:
                                    moff, mw = 256, 256
                                else:
                                    moff, mw = r * 128, 128
                                s_ps = psS.tile([128, 512], F32, tag="s")
                                nc.tensor.matmul(
                                    s_ps[:, off:],
                                    QQ[4][kt // 4][:, (kt % 4) * 128:
                                                   (kt % 4 + 1) * 128],
                                    QQ[h][b][:, off:],
                                    start=True, stop=True)
                                p_sb = ptp.tile([128, 512], MMDT, tag="p")
                                nc.scalar.activation(
                                    p_sb[:, off:], s_ps[:, off:],
                                    mybir.ActivationFunctionType.Exp,
                                    scale=SCALE)
                                if mw:
                                    nc.vector.tensor_tensor(
                                        p_sb[:, moff:moff + mw],
                                        p_sb[:, moff:moff + mw],
                                        MASK[:, r * 512 + moff:
                                             r * 512 + moff + mw],
                                        mybir.AluOpType.mult)
                                nc.tensor.matmul(
                                    y_ps[:, off:],
                                    V[kt // 4][:, (kt % 4) * 128:
                                               (kt % 4 + 1) * 128],
                                    p_sb[:, off:],
                                    start=(kt == 0), stop=(kt == nkt - 1))
                                nc.tensor.matmul(
                                    d_ps[:, off:], ONES[:, :1], p_sb[:, off:],
                                    start=(kt == 0), stop=(kt == nkt - 1))
                            recip = rcp.tile([1, 512], MMDT, tag="r")
                            with nc.allow_low_precision(
                                    reason="float32r is float32-width"):
                                nc.vector.reciprocal(recip[:], d_ps[:])
                            bc_ps = psB.tile([128, 512], F32, tag="bc")
                            nc.tensor.matmul(bc_ps[:], ONES[:1, :], recip[:],
                                             start=True, stop=True)
                            bc_sb = bcp.tile([128, 512], MMDT, tag="bc")
                            nc.vector.tensor_copy(bc_sb[:], bc_ps[:])
                            nc.vector.tensor_tensor(
                                Y[h][:, b * 512:(b + 1) * 512],
                                y_ps[:], bc_sb[:], mybir.AluOpType.mult)

                # ---------------- Phase C: output projection -------------
                with tc.tile_pool(name="outp", bufs=6) as outp, \
                     tc.tile_pool(name="psO", bufs=6, space="PSUM") as psO:
                    for tt in range(T // 128):
                        for cb in range(C // 512):
                            o_ps = psO.tile([128, 512], F32, tag="o")
                            for h in range(QPK):
                                nc.tensor.matmul(
                                    o_ps[:],
                                    Y[h][:, tt * 128:(tt + 1) * 128],
                                    WP[:, h * C + cb * 512:
                                       h * C + (cb + 1) * 512],
                                    start=(h == 0), stop=(h == QPK - 1))
                            o_sb = outp.tile([128, 512], F32, tag="o")
                            nc.scalar.copy(o_sb[:], o_ps[:])
                            nc.sync.dma_start(
                                d_out[tt * 128:(tt + 1) * 128,
                                      cb * 512:(cb + 1) * 512],
                                o_sb[:])
    nc.compile()
    return nc


def _host_inputs(x, cos, sin, w_attn, w_proj):
    """Build per-core input maps (host-side shard + transpose prep)."""
    f = np.float32
    xt = np.ascontiguousarray(x.reshape(T, C).T.astype(f))          # [C, T]
    cost = np.ascontiguousarray(cos.T.astype(f))                     # [HS, T]
    sgn = np.ones((HS, 1), f)
    sgn[:HS // 2] = -1.0
    sint = np.ascontiguousarray((sin.T * sgn).astype(f))             # signed sin
    # rot(x)=P@x in [d,t] layout; matmul computes lhsT.T @ rhs -> lhsT = P.T
    P = np.zeros((HS, HS), f)
    for i in range(HS // 2):
        P[i, i + HS // 2] = 1.0
        P[i + HS // 2, i] = 1.0
    perm = np.ascontiguousarray(P.T)
    idn = np.eye(128, dtype=f)
    ones = np.ones((128, 128), f)
    # causal mask tiles: keep iff qt_in_block >= r*128 + p
    fidx = np.arange(512)
    pidx = np.arange(128)
    mask = np.zeros((128, 4 * 512), f)
    for r in range(4):
        mask[:, r * 512:(r + 1) * 512] = (
            fidx[None, :] >= (r * 128 + pidx)[:, None]).astype(f)

    maps = []
    for g in range(N_CORES):
        wat = np.ascontiguousarray(
            w_attn[g * RG:(g + 1) * RG, :].T.astype(f))              # [C, RG]
        wpt = np.ascontiguousarray(
            w_proj[:, g * OG:(g + 1) * OG].T.astype(f))              # [OG, C]
        maps.append({
            "xt": xt, "wat": wat, "wpt": wpt, "cost": cost, "sint": sint,
            "mask": mask, "perm": perm, "idn": idn, "ones": ones,
        })
    return maps


_PROGRAM = None


def kernel(x, cos, sin, w_attn, w_proj):
    global _PROGRAM
    if _PROGRAM is None:
        _PROGRAM = _build_program()
    maps = _host_inputs(np.asarray(x), np.asarray(cos), np.asarray(sin),
                        np.asarray(w_attn), np.asarray(w_proj))
    res = bass_utils.run_bass_kernel_spmd(_PROGRAM, maps, list(range(N_CORES)))
    out = np.zeros((T, C), np.float32)
    for g in range(N_CORES):
        out += res.results[g]["out"]
    return out.reshape(1, T, C)

